# revision 14
# baseline (speedup 1.0000x reference)
"""Trainium2 Bass kernel for the quirky MultiHeadAttention module.

Reference computation (B=4, S=1024, H=768, NH=12, HS=64):
    Q = (x@Wq+bq)  split into heads     [B,12,S,64]
    K = (x@Wk+bk)  split into heads     [B,12,S,64]
    V = x@Wv+bv    NOT split            [B,S,768]
    A = softmax(QK^T/8 + mask)          [B,12,S,S]
    out = (A @ V) reshaped [B, S*12, H] @ Wo + bo    -> [4, 12288, 768]

Algebraic restructuring used here:
  * (A @ V) @ Wo = A @ (V @ Wo) = A @ (x @ (Wv@Wo) + 1x(bv@Wo)); with bo
    folded in, each output row is A[q,:] @ VW + c, c = bv@Wo + bo, and the
    +c term is realized exactly by adding a constant row to VW (softmax
    rows sum to one in exact correspondence with the sigma column below).
  * Masked keys produce exp(-1e9+s) == 0 in fp32 for every head and every
    query (the mask is [B,1,1,S]), identically in the reference, so masked
    keys are dropped entirely on the host and the key axis is compacted
    (~2x less attention work for a Bernoulli(1/2) mask).
  * The softmax denominator comes from a ones-column appended to VW, and
    exp needs no max-subtraction (scores are O(1) for this problem).

Sharding: 8 cores = 4 batches x 2 head-groups (6 heads each). Pure SPMD,
no collectives. Everything is computed in a transposed layout so no
on-device transposes are needed:
    QT/KT: [384 feat, tok] (head-pairs packed 64+64 in partitions; the
        64-row score matmuls are row-packed on the PE via tile_position)
    S^T = KT_h-slices.T @ QT_h  -> [k, q]  (k on partitions => the mask is
        a per-partition bias folded into the Exp activation for free)
    U = exp(S^T)  [k, q] fp16   -> exactly the layout the PV matmul needs
    O = U.T @ [VW | 1]  -> [q, 769] with col 768 = softmax denominator
Matmul operands are fp16 (same PE speed as bf16 on TRN2, ~4x less rounding
error); all accumulation is fp32 in PSUM. Inputs are packed host-side into
partition-major [128, N] blobs so the input DMAs run long contiguous lines,
and a burst of dummy matmuls during the initial DMA wait pre-warms the PE
clock (HAM) to 2.4 GHz.
"""

import math

import numpy as np

B, S, H, NH, HS = 4, 1024, 768, 12, 64
GW = 384          # head-group width = 6 heads * 64
NCORES = 8

_PROGRAM_CACHE = {}


def _pack6(a):
    """[768, N] -> partition-major [128, 6*N] (tile i at cols i*N:(i+1)*N)."""
    n = a.shape[1]
    return np.ascontiguousarray(
        a.reshape(6, 128, n).transpose(1, 0, 2).reshape(128, 6 * n))


def _build_program(kt_tiles, has_cvec):
    """kt_tiles: number of 128-wide compacted-key tiles (1..8).
    has_cvec: include the rank-1 (bv@Wo + bo) constant row in VW."""
    import concourse.mybir as mybir
    import concourse.tile as tile
    from concourse import bacc
    from concourse.bass import ds, ts

    f32 = mybir.dt.float32
    f16 = mybir.dt.float16
    AF = mybir.ActivationFunctionType

    KMAX = 128 * kt_tiles
    # key chunks (<=512 wide, balanced) for the KT projection
    if KMAX <= 512:
        kchunks = [(0, KMAX)]
    else:
        w1 = 128 * ((kt_tiles + 1) // 2)
        kchunks = [(0, w1), (w1, KMAX - w1)]

    nc = bacc.Bacc(None, target_bir_lowering=False, debug=False)

    xp_d = nc.dram_tensor("xp", (128, 6 * 1024), f16, kind="ExternalInput")
    wqp_d = nc.dram_tensor("wqp", (128, 6 * 384), f16, kind="ExternalInput")
    wkp_d = nc.dram_tensor("wkp", (128, 6 * 384), f16, kind="ExternalInput")
    wvp_d = nc.dram_tensor("wvp", (128, 6 * 768), f16, kind="ExternalInput")
    wvo6_d = nc.dram_tensor("wvo6", (1, 768), f16, kind="ExternalInput")
    # small fp32 per-partition vectors: cols = bq(3) bk(3) mk(kt_tiles)
    sv_d = nc.dram_tensor("sv", (128, 6 + kt_tiles), f32, kind="ExternalInput")
    out_d = nc.dram_tensor("out", (6, 1024, 768), f32, kind="ExternalOutput")

    with tile.TileContext(nc) as tc:
        with (
            tc.tile_pool(name="persist", bufs=1) as pp,
            tc.tile_pool(name="ut", bufs=4 * kt_tiles) as utp,
            tc.tile_pool(name="eps", bufs=8) as ep,
            tc.tile_pool(name="osb", bufs=4) as op_,
        ):
            # ---- stream inputs into SBUF (order = load priority) ----
            sv = pp.tile([128, 6 + kt_tiles], f32, name="sv", tag="sv")
            nc.sync.dma_start(sv[:], sv_d[:])
            bq_t = [sv[:, j:j + 1] for j in range(3)]
            bk_t = [sv[:, 3 + j:4 + j] for j in range(3)]
            mk_t = [sv[:, 6 + k:7 + k] for k in range(kt_tiles)]

            xbig = pp.tile([128, 6 * 1024], f16, name="xbig", tag="xbig")
            wqbig = pp.tile([128, 6 * 384], f16, name="wqbig", tag="wqbig")
            wkbig = pp.tile([128, 6 * 384], f16, name="wkbig", tag="wkbig")
            wvbig = pp.tile([128, 6 * 768], f16, name="wvbig", tag="wvbig")
            xkt6 = pp.tile([1, KMAX], f16, name="xkt6", tag="xkt6")
            wvo6 = pp.tile([1, 768], f16, name="wvo6", tag="wvo6")
            # Input loads: fine-grained pieces alternating over the two
            # HWDGE rings (sync, scalar) in consumption order, VW weights
            # on SWDGE (gpsimd). Small pieces land early so the first QT
            # matmuls can start while the rest of x streams in.
            rings = [nc.sync, nc.scalar]
            wh = 3 * 384
            for r in range(2):
                rings[r].dma_start(wqbig[:, r * wh:(r + 1) * wh],
                                   wqp_d[:, r * wh:(r + 1) * wh])
            for i in range(6):
                rings[i % 2].dma_start(xbig[:, i * 1024:(i + 1) * 1024],
                                       xp_d[:, i * 1024:(i + 1) * 1024])
            for r in range(2):
                rings[r].dma_start(wkbig[:, r * wh:(r + 1) * wh],
                                   wkp_d[:, r * wh:(r + 1) * wh])
            if has_cvec:
                nc.vector.memset(xkt6[:], 1.0)
                nc.scalar.dma_start(wvo6[:], wvo6_d[:])
            nc.gpsimd.dma_start(wvbig[:], wvp_d[:])

            xt = [xbig[:, i * 1024:(i + 1) * 1024] for i in range(6)]
            wq_t = [wqbig[:, i * 384:(i + 1) * 384] for i in range(6)]
            # tokens are host-permuted (kept keys first), so the K-side
            # tiles are just the leading columns of the same x buffer
            xkt = [xbig[:, i * 1024:i * 1024 + KMAX] for i in range(6)]
            wk_t = [wkbig[:, i * 384:(i + 1) * 384] for i in range(6)]
            wvo_t = [wvbig[:, i * 768:(i + 1) * 768] for i in range(6)]

            # persistent intermediates
            QT = [pp.tile([128, 1024], f16, name=f"QT{j}", tag=f"QT{j}")
                  for j in range(3)]
            KT = [pp.tile([128, KMAX], f16, name=f"KT{j}", tag=f"KT{j}")
                  for j in range(3)]
            VW = [pp.tile([128, 769], f16, name=f"VW{m}", tag=f"VW{m}")
                  for m in range(kt_tiles)]

            # ---- phase A: projections ----
            # PE warm-up: dummy matmuls on a tiny memset tile keep the
            # tensor engine active during the initial input DMA so the
            # HAM clock gate opens (2.4 GHz) before real work arrives.
            wsrc = pp.tile([1, 512], f16, name="wsrc", tag="wsrc")
            nc.vector.memset(wsrc[:], 0.0)
            with tc.tile_pool(name="psW", bufs=2, space="PSUM") as psW:
                for _ in range(34):
                    psw = psW.tile([1, 512], f32, name="warm", tag="warm")
                    nc.tensor.matmul(psw[:], wsrc[:, 0:1], wsrc[:])

            with tc.tile_pool(name="psA", bufs=6, space="PSUM") as psA:
                # QT is kt-major: all six (j,qc) PSUM groups accumulate in
                # parallel so each arriving x tile is consumed immediately
                # (no long PE stalls while x streams in).
                qgroups = [(j, qc) for j in range(3) for qc in range(2)]
                qps = [psA.tile([128, 512], f32, name=f"qtp{j}{qc}", tag="qk")
                       for j, qc in qgroups]
                for kt in range(6):
                    for gi, (j, qc) in enumerate(qgroups):
                        nc.tensor.matmul(
                            qps[gi][:], wq_t[kt][:, ts(j, 128)],
                            xt[kt][:, ds(qc * 512, 512)],
                            start=(kt == 0), stop=(kt == 5))
                for gi, (j, qc) in enumerate(qgroups):
                    nc.scalar.activation(
                        QT[j][:, ds(qc * 512, 512)], qps[gi][:], AF.Identity,
                        bias=bq_t[j])
                for j in range(3):
                    for o, w in kchunks:
                        kch = ds(o, w)
                        ps2 = psA.tile([128, 512], f32, name="ktp", tag="qk")
                        for kt in range(6):
                            nc.tensor.matmul(
                                ps2[:, 0:w], wk_t[kt][:, ts(j, 128)],
                                xkt[kt][:, kch],
                                start=(kt == 0), stop=(kt == 5))
                        nc.scalar.activation(
                            KT[j][:, kch], ps2[:, 0:w], AF.Identity,
                            bias=bk_t[j])
                for m in range(kt_tiles):   # compacted-key token tile
                    for ncn in range(2):    # output feature chunk of 384
                        fch = ds(ncn * 384, 384)
                        ps = psA.tile([128, 384], f32, name="vw", tag="vw",
                                      bufs=2)
                        for kt in range(6):
                            nc.tensor.matmul(
                                ps[:], xkt[kt][:, ts(m, 128)], wvo_t[kt][:, fch],
                                start=(kt == 0),
                                stop=(kt == 5 and not has_cvec))
                        if has_cvec:
                            nc.tensor.matmul(
                                ps[:], xkt6[:, ts(m, 128)], wvo6[:, fch],
                                start=False, stop=True)
                        nc.vector.tensor_copy(VW[m][:, fch], ps[:])
                    nc.vector.memset(VW[m][:, 768:769], 1.0)

            # ---- phase B: attention ----
            with (
                tc.tile_pool(name="psS", bufs=4, space="PSUM") as psSp,
                tc.tile_pool(name="psO", bufs=2, space="PSUM") as psOp,
            ):
                for j in range(3):
                    for qc in range(2):
                        qch = ds(qc * 512, 512)
                        ut = [[None] * kt_tiles for _ in range(2)]
                        for kt in range(kt_tiles):
                            for hh in range(2):
                                p0 = hh * 64
                                ps = psSp.tile([128, 512], f32, name="psS",
                                               tag="psS")
                                # 64-row-packed scores^T: [k-tile, q-chunk]
                                nc.tensor.matmul(
                                    ps[:],
                                    KT[j][p0:p0 + 64, ts(kt, 128)],
                                    QT[j][p0:p0 + 64, qch])
                                u = utp.tile([128, 512], f16, name="ut",
                                             tag="ut")
                                nc.scalar.activation(
                                    u[:], ps[:], AF.Exp, bias=mk_t[kt])
                                ut[hh][kt] = u
                        for hh in range(2):
                            head = j * 2 + hh
                            for mq in range(4):
                                pa = psOp.tile([128, 512], f32, name="psOa",
                                               tag="psOa")
                                pb = psOp.tile([128, 257], f32, name="psOb",
                                               tag="psOb")
                                for kt in range(kt_tiles):
                                    nc.tensor.matmul(
                                        pa[:], ut[hh][kt][:, ts(mq, 128)],
                                        VW[kt][:, 0:512],
                                        start=(kt == 0),
                                        stop=(kt == kt_tiles - 1))
                                for kt in range(kt_tiles):
                                    nc.tensor.matmul(
                                        pb[:], ut[hh][kt][:, ts(mq, 128)],
                                        VW[kt][:, 512:769],
                                        start=(kt == 0),
                                        stop=(kt == kt_tiles - 1))
                                rv = ep.tile([128, 1], f32, name="rinv",
                                             tag="rinv")
                                nc.vector.reciprocal(rv[:], pb[:, 256:257])
                                ob = op_.tile([128, 768], f32, name="ob",
                                              tag="ob")
                                orow = out_d[head, ds(qc * 512 + mq * 128, 128), :]
                                nc.vector.tensor_scalar_mul(
                                    ob[:, 0:512], pa[:], rv[:])
                                nc.sync.dma_start(orow[:, 0:512], ob[:, 0:512])
                                nc.vector.tensor_scalar_mul(
                                    ob[:, 512:768], pb[:, 0:256], rv[:])
                                nc.scalar.dma_start(orow[:, 512:768],
                                                    ob[:, 512:768])
    nc.compile()
    return nc


def get_program(kt_tiles=8, has_cvec=True):
    key = (kt_tiles, has_cvec)
    if key not in _PROGRAM_CACHE:
        _PROGRAM_CACHE[key] = _build_program(*key)
    return _PROGRAM_CACHE[key]


def prep(x, mask, Wq, bq, Wk, bk, Wv, bv, Wo, bo):
    """Host-side sharding/compaction.
    Tokens are permuted per batch so unmasked keys come first; the device
    computes everything in permuted token order and gather_output undoes
    the permutation. Returns (kt_tiles, has_cvec, in_maps, perms)."""
    f16 = np.float16
    x = np.asarray(x, np.float32)
    mask = np.asarray(mask)
    Wq = np.asarray(Wq, np.float32)
    Wk = np.asarray(Wk, np.float32)
    Wv = np.asarray(Wv, np.float32)
    Wo = np.asarray(Wo, np.float32)
    bq = np.asarray(bq, np.float32)
    bk = np.asarray(bk, np.float32)
    bv = np.asarray(bv, np.float32)
    bo = np.asarray(bo, np.float32)

    mrow = [mask[b, 0, 0] != 0 for b in range(B)]
    perms = [np.argsort(~mrow[b], kind="stable") for b in range(B)]
    nkeep = [int(mrow[b].sum()) for b in range(B)]
    kt_tiles = min(8, max(1, math.ceil(max(nkeep) / 128)))
    KMAX = 128 * kt_tiles

    cvec = bv @ Wo + bo
    has_cvec = bool(np.any(cvec))

    # per-head-group packed weights (shared across the 4 batches)
    wq_p, wk_p, bq_p, bk_p = [], [], [], []
    for g in range(2):
        cs = slice(g * GW, (g + 1) * GW)
        wq_p.append(_pack6((Wq[:, cs] * 0.125).astype(f16)))
        wk_p.append(_pack6(Wk[:, cs].astype(f16)))
        bq_p.append((bq[cs] * 0.125).reshape(3, 128).T)   # [128,3]
        bk_p.append(bk[cs].reshape(3, 128).T)
    wvp = _pack6((Wv @ Wo).astype(f16))
    wvo6 = cvec.astype(f16).reshape(1, 768)

    xp_b, sv_b = [], []
    for b in range(B):
        xp_b.append(_pack6(x[b][perms[b]].T.astype(f16)))
        sv = np.empty((128, 6 + kt_tiles), np.float32)
        mk = np.full(KMAX, -1e9, np.float32)
        mk[:nkeep[b]] = 0.0
        sv[:, 6:] = mk.reshape(kt_tiles, 128).T
        sv_b.append(sv)

    in_maps = []
    for c in range(NCORES):
        b, g = c // 2, c % 2
        sv = sv_b[b].copy()
        sv[:, 0:3] = bq_p[g]
        sv[:, 3:6] = bk_p[g]
        in_maps.append({
            "xp": xp_b[b],
            "wqp": wq_p[g],
            "wkp": wk_p[g],
            "wvp": wvp,
            "wvo6": wvo6,
            "sv": sv,
        })
    return kt_tiles, has_cvec, in_maps, perms


def gather_output(results, perms):
    out = np.empty((B, S * NH, H), np.float32)
    ov = out.reshape(B, S, NH, H)
    for c in range(NCORES):
        b, g = c // 2, c % 2
        o = results[c]["out"]  # [6, 1024(permuted q), 768]
        ov[b, perms[b], g * 6:(g + 1) * 6, :] = o.transpose(1, 0, 2)
    return out


def kernel(**inputs):
    from concourse.bass_utils import run_bass_kernel_spmd

    kt_tiles, has_cvec, in_maps, perms = prep(**inputs)
    nc = get_program(kt_tiles, has_cvec)
    res = run_bass_kernel_spmd(nc, in_maps, core_ids=list(range(NCORES)))
    return gather_output(res.results, perms)


if __name__ == "__main__":
    rng = np.random.default_rng(0)
    demo = {
        "x": rng.standard_normal((B, S, H), dtype=np.float32),
        "mask": rng.integers(0, 2, (B, 1, 1, S)).astype(np.int32),
        "Wq": rng.standard_normal((H, H), dtype=np.float32) / np.sqrt(H),
        "bq": np.zeros(H, np.float32),
        "Wk": rng.standard_normal((H, H), dtype=np.float32) / np.sqrt(H),
        "bk": np.zeros(H, np.float32),
        "Wv": rng.standard_normal((H, H), dtype=np.float32) / np.sqrt(H),
        "bv": np.zeros(H, np.float32),
        "Wo": rng.standard_normal((H, H), dtype=np.float32) / np.sqrt(H),
        "bo": np.zeros(H, np.float32),
    }
    out = kernel(**demo)
    print("kernel ran, output shape", out.shape)


# revision 17
# speedup vs baseline: 1.0110x; 1.0110x over previous
"""Trainium2 Bass kernel for the quirky MultiHeadAttention module.

Reference computation (B=4, S=1024, H=768, NH=12, HS=64):
    Q = (x@Wq+bq)  split into heads     [B,12,S,64]
    K = (x@Wk+bk)  split into heads     [B,12,S,64]
    V = x@Wv+bv    NOT split            [B,S,768]
    A = softmax(QK^T/8 + mask)          [B,12,S,S]
    out = (A @ V) reshaped [B, S*12, H] @ Wo + bo    -> [4, 12288, 768]

Algebraic restructuring used here:
  * (A @ V) @ Wo = A @ (V @ Wo) = A @ (x @ (Wv@Wo) + 1x(bv@Wo)); with bo
    folded in, each output row is A[q,:] @ VW + c, c = bv@Wo + bo, and the
    +c term is realized exactly by adding a constant row to VW (softmax
    rows sum to one in exact correspondence with the sigma column below).
  * Masked keys produce exp(-1e9+s) == 0 in fp32 for every head and every
    query (the mask is [B,1,1,S]), identically in the reference, so masked
    keys are dropped entirely on the host and the key axis is compacted
    (~2x less attention work for a Bernoulli(1/2) mask).
  * The softmax denominator comes from a ones-column appended to VW, and
    exp needs no max-subtraction (scores are O(1) for this problem).

Sharding: 8 cores = 4 batches x 2 head-groups (6 heads each). Pure SPMD,
no collectives. Everything is computed in a transposed layout so no
on-device transposes are needed:
    QT/KT: [384 feat, tok] (head-pairs packed 64+64 in partitions; the
        64-row score matmuls are row-packed on the PE via tile_position)
    S^T = KT_h-slices.T @ QT_h  -> [k, q]  (k on partitions => the mask is
        a per-partition bias folded into the Exp activation for free)
    U = exp(S^T)  [k, q] fp16   -> exactly the layout the PV matmul needs
    O = U.T @ [VW | 1]  -> [q, 769] with col 768 = softmax denominator
Matmul operands are fp16 (same PE speed as bf16 on TRN2, ~4x less rounding
error); all accumulation is fp32 in PSUM. Inputs are packed host-side into
partition-major [128, N] blobs so the input DMAs run long contiguous lines,
and a burst of dummy matmuls during the initial DMA wait pre-warms the PE
clock (HAM) to 2.4 GHz.
"""

import math

import numpy as np

B, S, H, NH, HS = 4, 1024, 768, 12, 64
GW = 384          # head-group width = 6 heads * 64
NCORES = 8

_PROGRAM_CACHE = {}


def _pack6(a):
    """[768, N] -> partition-major [128, 6*N] (tile i at cols i*N:(i+1)*N)."""
    n = a.shape[1]
    return np.ascontiguousarray(
        a.reshape(6, 128, n).transpose(1, 0, 2).reshape(128, 6 * n))


def _build_program(kt_tiles, has_cvec):
    """kt_tiles: number of 128-wide compacted-key tiles (1..8).
    has_cvec: include the rank-1 (bv@Wo + bo) constant row in VW."""
    import concourse.mybir as mybir
    import concourse.tile as tile
    from concourse import bacc
    from concourse.bass import ds, ts

    f32 = mybir.dt.float32
    f16 = mybir.dt.float16
    AF = mybir.ActivationFunctionType

    KMAX = 128 * kt_tiles
    # key chunks (<=512 wide, balanced) for the KT projection
    if KMAX <= 512:
        kchunks = [(0, KMAX)]
    else:
        w1 = 128 * ((kt_tiles + 1) // 2)
        kchunks = [(0, w1), (w1, KMAX - w1)]

    nc = bacc.Bacc(None, target_bir_lowering=False, debug=False)

    xp_d = nc.dram_tensor("xp", (128, 6 * 1024), f16, kind="ExternalInput")
    wqp_d = nc.dram_tensor("wqp", (128, 6 * 384), f16, kind="ExternalInput")
    wkp_d = nc.dram_tensor("wkp", (128, 6 * 384), f16, kind="ExternalInput")
    wvp_d = nc.dram_tensor("wvp", (128, 6 * 768), f16, kind="ExternalInput")
    wvo6_d = nc.dram_tensor("wvo6", (1, 768), f16, kind="ExternalInput")
    # small fp32 per-partition vectors: cols = bq(3) bk(3) mk(kt_tiles)
    sv_d = nc.dram_tensor("sv", (128, 6 + kt_tiles), f32, kind="ExternalInput")
    out_d = nc.dram_tensor("out", (6, 1024, 768), f32, kind="ExternalOutput")

    with tile.TileContext(nc) as tc:
        with (
            tc.tile_pool(name="persist", bufs=1) as pp,
            tc.tile_pool(name="ut", bufs=4 * kt_tiles) as utp,
            tc.tile_pool(name="eps", bufs=8) as ep,
            tc.tile_pool(name="osb", bufs=4) as op_,
        ):
            # ---- stream inputs into SBUF (order = load priority) ----
            sv = pp.tile([128, 6 + kt_tiles], f32, name="sv", tag="sv")
            nc.sync.dma_start(sv[:], sv_d[:])
            bq_t = [sv[:, j:j + 1] for j in range(3)]
            bk_t = [sv[:, 3 + j:4 + j] for j in range(3)]
            mk_t = [sv[:, 6 + k:7 + k] for k in range(kt_tiles)]

            xbig = pp.tile([128, 6 * 1024], f16, name="xbig", tag="xbig")
            wqbig = pp.tile([128, 6 * 384], f16, name="wqbig", tag="wqbig")
            wkbig = pp.tile([128, 6 * 384], f16, name="wkbig", tag="wkbig")
            wvbig = pp.tile([128, 6 * 768], f16, name="wvbig", tag="wvbig")
            xkt6 = pp.tile([1, KMAX], f16, name="xkt6", tag="xkt6")
            wvo6 = pp.tile([1, 768], f16, name="wvo6", tag="wvo6")
            # Input loads: fine-grained pieces alternating over the two
            # HWDGE rings (sync, scalar) in consumption order, VW weights
            # on SWDGE (gpsimd). Small pieces land early so the first QT
            # matmuls can start while the rest of x streams in.
            rings = [nc.sync, nc.scalar]
            wh = 3 * 384
            for r in range(2):
                rings[r].dma_start(wqbig[:, r * wh:(r + 1) * wh],
                                   wqp_d[:, r * wh:(r + 1) * wh])
            for i in range(6):
                rings[i % 2].dma_start(xbig[:, i * 1024:(i + 1) * 1024],
                                       xp_d[:, i * 1024:(i + 1) * 1024])
            for r in range(2):
                rings[r].dma_start(wkbig[:, r * wh:(r + 1) * wh],
                                   wkp_d[:, r * wh:(r + 1) * wh])
            if has_cvec:
                nc.vector.memset(xkt6[:], 1.0)
                nc.scalar.dma_start(wvo6[:], wvo6_d[:])
            nc.gpsimd.dma_start(wvbig[:], wvp_d[:])

            xt = [xbig[:, i * 1024:(i + 1) * 1024] for i in range(6)]
            wq_t = [wqbig[:, i * 384:(i + 1) * 384] for i in range(6)]
            # tokens are host-permuted (kept keys first), so the K-side
            # tiles are just the leading columns of the same x buffer
            xkt = [xbig[:, i * 1024:i * 1024 + KMAX] for i in range(6)]
            wk_t = [wkbig[:, i * 384:(i + 1) * 384] for i in range(6)]
            wvo_t = [wvbig[:, i * 768:(i + 1) * 768] for i in range(6)]

            # persistent intermediates
            QT = [pp.tile([128, 1024], f16, name=f"QT{j}", tag=f"QT{j}")
                  for j in range(3)]
            KT = [pp.tile([128, KMAX], f16, name=f"KT{j}", tag=f"KT{j}")
                  for j in range(3)]
            VW = [pp.tile([128, 769], f16, name=f"VW{m}", tag=f"VW{m}")
                  for m in range(kt_tiles)]

            # ---- phase A: projections ----
            # PE warm-up: dummy matmuls on a tiny memset tile keep the
            # tensor engine active during the initial input DMA so the
            # HAM clock gate opens (2.4 GHz) before real work arrives.
            wsrc = pp.tile([1, 512], f16, name="wsrc", tag="wsrc")
            nc.vector.memset(wsrc[:], 0.0)
            with tc.tile_pool(name="psW", bufs=2, space="PSUM") as psW:
                for _ in range(12):
                    psw = psW.tile([1, 512], f32, name="warm", tag="warm")
                    nc.tensor.matmul(psw[:], wsrc[:, 0:1], wsrc[:])

            with tc.tile_pool(name="psA", bufs=6, space="PSUM") as psA:
                # QT is kt-major: all six (j,qc) PSUM groups accumulate in
                # parallel so each arriving x tile is consumed immediately
                # (no long PE stalls while x streams in).
                qgroups = [(j, qc) for j in range(3) for qc in range(2)]
                qps = [psA.tile([128, 512], f32, name=f"qtp{j}{qc}", tag="qk")
                       for j, qc in qgroups]
                for kt in range(6):
                    for gi, (j, qc) in enumerate(qgroups):
                        nc.tensor.matmul(
                            qps[gi][:], wq_t[kt][:, ts(j, 128)],
                            xt[kt][:, ds(qc * 512, 512)],
                            start=(kt == 0), stop=(kt == 5))
                for gi, (j, qc) in enumerate(qgroups):
                    nc.scalar.activation(
                        QT[j][:, ds(qc * 512, 512)], qps[gi][:], AF.Identity,
                        bias=bq_t[j])
                for j in range(3):
                    for o, w in kchunks:
                        kch = ds(o, w)
                        ps2 = psA.tile([128, 512], f32, name="ktp", tag="qk")
                        for kt in range(6):
                            nc.tensor.matmul(
                                ps2[:, 0:w], wk_t[kt][:, ts(j, 128)],
                                xkt[kt][:, kch],
                                start=(kt == 0), stop=(kt == 5))
                        nc.scalar.activation(
                            KT[j][:, kch], ps2[:, 0:w], AF.Identity,
                            bias=bk_t[j])
                for m in range(kt_tiles):   # compacted-key token tile
                    for ncn in range(2):    # output feature chunk of 384
                        fch = ds(ncn * 384, 384)
                        ps = psA.tile([128, 384], f32, name="vw", tag="vw",
                                      bufs=2)
                        for kt in range(6):
                            nc.tensor.matmul(
                                ps[:], xkt[kt][:, ts(m, 128)], wvo_t[kt][:, fch],
                                start=(kt == 0),
                                stop=(kt == 5 and not has_cvec))
                        if has_cvec:
                            nc.tensor.matmul(
                                ps[:], xkt6[:, ts(m, 128)], wvo6[:, fch],
                                start=False, stop=True)
                        nc.vector.tensor_copy(VW[m][:, fch], ps[:])
                    nc.vector.memset(VW[m][:, 768:769], 1.0)

            # ---- phase B: attention ----
            with (
                tc.tile_pool(name="psS", bufs=4, space="PSUM") as psSp,
                tc.tile_pool(name="psO", bufs=2, space="PSUM") as psOp,
            ):
                chunks = [(j, qc) for j in range(3) for qc in range(2)]

                def emit_scores(j, qc):
                    qch = ds(qc * 512, 512)
                    ut = [[None] * kt_tiles for _ in range(2)]
                    for kt in range(kt_tiles):
                        for hh in range(2):
                            p0 = hh * 64
                            ps = psSp.tile([128, 512], f32, name="psS",
                                           tag="psS")
                            # 64-row-packed scores^T: [k-tile, q-chunk]
                            nc.tensor.matmul(
                                ps[:],
                                KT[j][p0:p0 + 64, ts(kt, 128)],
                                QT[j][p0:p0 + 64, qch])
                            u = utp.tile([128, 512], f16, name="ut", tag="ut")
                            nc.scalar.activation(
                                u[:], ps[:], AF.Exp, bias=mk_t[kt])
                            ut[hh][kt] = u
                    return ut

                for ci, (j, qc) in enumerate(chunks):
                    ut = emit_scores(j, qc)
                    for gi, (hh, mq) in enumerate(
                            (hh, mq) for hh in range(2) for mq in range(4)):
                        head = j * 2 + hh
                        pa = psOp.tile([128, 512], f32, name="psOa",
                                       tag="psOa")
                        pb = psOp.tile([128, 257], f32, name="psOb",
                                       tag="psOb")
                        for kt in range(kt_tiles):
                            nc.tensor.matmul(
                                pa[:], ut[hh][kt][:, ts(mq, 128)],
                                VW[kt][:, 0:512],
                                start=(kt == 0), stop=(kt == kt_tiles - 1))
                        for kt in range(kt_tiles):
                            nc.tensor.matmul(
                                pb[:], ut[hh][kt][:, ts(mq, 128)],
                                VW[kt][:, 512:769],
                                start=(kt == 0), stop=(kt == kt_tiles - 1))
                        rv = ep.tile([128, 1], f32, name="rinv", tag="rinv")
                        nc.vector.reciprocal(rv[:], pb[:, 256:257])
                        ob = op_.tile([128, 768], f32, name="ob", tag="ob")
                        orow = out_d[head, ds(qc * 512 + mq * 128, 128), :]
                        nc.vector.tensor_scalar_mul(
                            ob[:, 0:512], pa[:], rv[:])
                        nc.sync.dma_start(orow[:, 0:512], ob[:, 0:512])
                        nc.vector.tensor_scalar_mul(
                            ob[:, 512:768], pb[:, 0:256], rv[:])
                        nc.scalar.dma_start(orow[:, 512:768], ob[:, 512:768])
    nc.compile()
    return nc


def get_program(kt_tiles=8, has_cvec=True):
    key = (kt_tiles, has_cvec)
    if key not in _PROGRAM_CACHE:
        _PROGRAM_CACHE[key] = _build_program(*key)
    return _PROGRAM_CACHE[key]


def prep(x, mask, Wq, bq, Wk, bk, Wv, bv, Wo, bo):
    """Host-side sharding/compaction.
    Tokens are permuted per batch so unmasked keys come first; the device
    computes everything in permuted token order and gather_output undoes
    the permutation. Returns (kt_tiles, has_cvec, in_maps, perms)."""
    f16 = np.float16
    x = np.asarray(x, np.float32)
    mask = np.asarray(mask)
    Wq = np.asarray(Wq, np.float32)
    Wk = np.asarray(Wk, np.float32)
    Wv = np.asarray(Wv, np.float32)
    Wo = np.asarray(Wo, np.float32)
    bq = np.asarray(bq, np.float32)
    bk = np.asarray(bk, np.float32)
    bv = np.asarray(bv, np.float32)
    bo = np.asarray(bo, np.float32)

    mrow = [mask[b, 0, 0] != 0 for b in range(B)]
    perms = [np.argsort(~mrow[b], kind="stable") for b in range(B)]
    nkeep = [int(mrow[b].sum()) for b in range(B)]
    kt_tiles = min(8, max(1, math.ceil(max(nkeep) / 128)))
    KMAX = 128 * kt_tiles

    cvec = bv @ Wo + bo
    has_cvec = bool(np.any(cvec))

    # per-head-group packed weights (shared across the 4 batches)
    wq_p, wk_p, bq_p, bk_p = [], [], [], []
    for g in range(2):
        cs = slice(g * GW, (g + 1) * GW)
        wq_p.append(_pack6((Wq[:, cs] * 0.125).astype(f16)))
        wk_p.append(_pack6(Wk[:, cs].astype(f16)))
        bq_p.append((bq[cs] * 0.125).reshape(3, 128).T)   # [128,3]
        bk_p.append(bk[cs].reshape(3, 128).T)
    wvp = _pack6((Wv @ Wo).astype(f16))
    wvo6 = cvec.astype(f16).reshape(1, 768)

    xp_b, sv_b = [], []
    for b in range(B):
        xp_b.append(_pack6(x[b][perms[b]].T.astype(f16)))
        sv = np.empty((128, 6 + kt_tiles), np.float32)
        mk = np.full(KMAX, -1e9, np.float32)
        mk[:nkeep[b]] = 0.0
        sv[:, 6:] = mk.reshape(kt_tiles, 128).T
        sv_b.append(sv)

    in_maps = []
    for c in range(NCORES):
        b, g = c // 2, c % 2
        sv = sv_b[b].copy()
        sv[:, 0:3] = bq_p[g]
        sv[:, 3:6] = bk_p[g]
        in_maps.append({
            "xp": xp_b[b],
            "wqp": wq_p[g],
            "wkp": wk_p[g],
            "wvp": wvp,
            "wvo6": wvo6,
            "sv": sv,
        })
    return kt_tiles, has_cvec, in_maps, perms


def gather_output(results, perms):
    out = np.empty((B, S * NH, H), np.float32)
    ov = out.reshape(B, S, NH, H)
    for c in range(NCORES):
        b, g = c // 2, c % 2
        o = results[c]["out"]  # [6, 1024(permuted q), 768]
        ov[b, perms[b], g * 6:(g + 1) * 6, :] = o.transpose(1, 0, 2)
    return out


def kernel(**inputs):
    from concourse.bass_utils import run_bass_kernel_spmd

    kt_tiles, has_cvec, in_maps, perms = prep(**inputs)
    nc = get_program(kt_tiles, has_cvec)
    res = run_bass_kernel_spmd(nc, in_maps, core_ids=list(range(NCORES)))
    return gather_output(res.results, perms)


if __name__ == "__main__":
    rng = np.random.default_rng(0)
    demo = {
        "x": rng.standard_normal((B, S, H), dtype=np.float32),
        "mask": rng.integers(0, 2, (B, 1, 1, S)).astype(np.int32),
        "Wq": rng.standard_normal((H, H), dtype=np.float32) / np.sqrt(H),
        "bq": np.zeros(H, np.float32),
        "Wk": rng.standard_normal((H, H), dtype=np.float32) / np.sqrt(H),
        "bk": np.zeros(H, np.float32),
        "Wv": rng.standard_normal((H, H), dtype=np.float32) / np.sqrt(H),
        "bv": np.zeros(H, np.float32),
        "Wo": rng.standard_normal((H, H), dtype=np.float32) / np.sqrt(H),
        "bo": np.zeros(H, np.float32),
    }
    out = kernel(**demo)
    print("kernel ran, output shape", out.shape)
